# revision 33
# baseline (speedup 1.0000x reference)
"""Trainium2 Bass kernel for the collaborative attention layer.

Shapes (hardcoded): B=2, L=S=2048, DIN=DK=DV=DOUT=512, H=8.
Sharding over 8 cores: core i handles batch b=i//4 and head pair
{2*(i%4), 2*(i%4)+1} (data-parallel on B, tensor-parallel on H).

Per-core device program (SPMD, identical program, different shards):
  kT   = Wk @ keys.T          [e, s]   (fp32r matmuls, contract d on partitions)
  cbT  = Wcb @ keys.T / 8     [2, s]  -> PE-transposed into per-partition layout
  v    = values @ Wv.T        [s, v]   plus an appended ones column per head
  per l-group of 512, per head:
    qT chunk = Wq @ queries.T [e, l]
    mqT      = qT * mixing_h  (per-partition scalar)
    scoresT  = kT.T-tiles @ mqT -> PSUM [s_tile, l]
    E        = exp(scoresT/8 + cb_h/8)   (ACT, bias per partition)
    ctxT_u/Z = [v_h | 1].T @ E  -> PSUM [65, l]  (row 64 = softmax denom Z)
    R        = 1/Z; Rbc = ones.T @ R (PE rank-1 broadcast)
    probsT   = E * Rbc  -> DMA out (transposed layout, host fixes with a view)
    ctxT_n   = ctxT_u * Rbc
  outT partial = Wd_pair.T-tiles @ [ctxT_n(h0); ctxT_n(h1)] -> DMA out

Host adds the per-batch partials, bd, and Wd @ bv (bias algebra: adding bv to v
adds (Wd @ bv) to out because softmax rows sum to 1), and transposes views.
"""

import os
import sys

sys.path.insert(0, "/opt/trn_rl_repo")

import ml_dtypes
import numpy as np

import concourse.bass as bass
import concourse.mybir as mybir
import concourse.tile as tile
from concourse.bass_utils import run_bass_kernel_spmd
from concourse.masks import make_identity
from concourse.vector_clock import ScopedClock

B, L, S, D, H = 2, 2048, 2048, 512, 8
HD = D // H          # 64 per-head value dim
PAIRS = 4            # head pairs per batch
F32 = mybir.dt.float32
F32R = mybir.dt.float32r
BF16 = mybir.dt.bfloat16
INV_SQRT_HEAD = 1.0 / 8.0  # 1/sqrt(DK/H) = 1/sqrt(64)

DT = D // 128        # 4 contraction tiles of 128
ST = S // 128        # 16 key/value tiles of 128
LG = 4               # l groups
LW = L // LG         # 512 moving width


def _split_multi_waits(nc: bass.Bass) -> None:
    """This walrus build rejects instructions carrying more than one sync
    wait (CoreV3 setupSyncWait). Rewrite any multi-wait instruction into
    single-wait EventSemaphore carriers (what wait_ge lowers to) followed
    by the original instruction with its last wait - identical semantics,
    since waits on the same engine queue AND sequentially."""
    uid = 0
    for f in nc.m.functions:
        for bb in f.blocks:
            insts = bb.instructions
            i = 0
            while i < len(insts):
                inst = insts[i]
                si = inst.sync_info
                if si is not None and si.on_wait and len(si.on_wait) > 1:
                    waits = list(si.on_wait)
                    si.on_wait = waits[-1:]
                    for w in waits[:-1]:
                        carrier = mybir.InstEventSemaphore(
                            name=f"waitsplit-{uid}", ins=[], outs=[]
                        )
                        uid += 1
                        carrier.engine = inst.engine
                        carrier.sync_info = mybir.SyncInfo(on_wait=[w], on_update=[])
                        insts.insert(i, carrier)
                        i += 1
                i += 1


class _SplitDrainTileContext(tile.TileContext):
    """Kept as a plain alias; multi-wait splitting happens in
    _split_multi_waits after the TileContext exits."""


def _r(ap):
    return ap.bitcast(F32R)


def _build_program() -> bass.Bass:
    nc = bass.Bass()

    # Matmul-fed inputs are declared float32r (same bits as f32; the BIR
    # verifier requires fp32r matmul operands to come from fp32r-typed
    # producers).
    qT_d = nc.declare_dram_parameter("qT", [D, L], F32R, isOutput=False)
    kT_d = nc.declare_dram_parameter("kTin", [D, S], F32R, isOutput=False)
    vT_d = nc.declare_dram_parameter("vTin", [D, S], F32R, isOutput=False)
    WqT_d = nc.declare_dram_parameter("WqT", [D, D], F32R, isOutput=False)
    WkT_d = nc.declare_dram_parameter("WkT", [D, D], F32R, isOutput=False)
    WvT_d = nc.declare_dram_parameter("WvT", [D, 256], F32R, isOutput=False)
    WcbT_d = nc.declare_dram_parameter("WcbT", [D, 2], F32R, isOutput=False)
    mixT_d = nc.declare_dram_parameter("mixT", [D, 2], F32, isOutput=False)
    WdT_d = nc.declare_dram_parameter("WdT", [2, HD, D], F32R, isOutput=False)
    ones_d = nc.declare_dram_parameter("ones", [128, 128], F32R, isOutput=False)

    probsT_d = nc.declare_dram_parameter("probsT", [2, S, L], F32, isOutput=True)
    outT_d = nc.declare_dram_parameter("outT", [D, L], F32, isOutput=True)

    # d-major tiled views: row d = t*128 + p  ->  [p, t, n]
    qT_v = qT_d.rearrange("(t p) n -> p t n", p=128)
    kT_v = kT_d.rearrange("(t p) n -> p t n", p=128)
    vT_v = vT_d.rearrange("(t p) n -> p t n", p=128)
    WqT_v = WqT_d.rearrange("(t p) n -> p t n", p=128)
    WkT_v = WkT_d.rearrange("(t p) n -> p t n", p=128)
    WvT_v = WvT_d.rearrange("(t p) n -> p t n", p=128)
    WcbT_v = WcbT_d.rearrange("(t p) n -> p t n", p=128)
    mixT_v = mixT_d.rearrange("(t p) n -> p t n", p=128)

    with tile.TileContext(nc) as tc:
        with (
            tc.tile_pool(name="persist", bufs=1) as persist,
            tc.tile_pool(name="main", bufs=2) as main,
            tc.tile_pool(name="epool", bufs=1) as epool,
            tc.tile_pool(name="stage", bufs=4) as stage,
        ):
            # ---------------- persistent tiles ----------------
            kT_sb = persist.tile([128, DT, S], F32R)      # 32KB/part
            v_sb = persist.tile([128, ST, 132], BF16)     # [s, v'] per head pair
            WqT_sb = persist.tile([128, DT, D], F32R)
            WdT0_sb = persist.tile([HD, D], F32R)
            WdT1_sb = persist.tile([HD, D], F32R)
            mixT_sb = persist.tile([128, DT, 2], F32)
            cb8_sb = persist.tile([128, ST, 2], F32)      # cb/8, s on partitions
            ones_sb = persist.tile([128, 128], F32R)
            id2_sb = persist.tile([2, 2], F32)

            nc.sync.dma_start(out=WqT_sb[:], in_=WqT_v[:])
            nc.sync.dma_start(out=WdT0_sb[:], in_=WdT_d[0])
            nc.sync.dma_start(out=WdT1_sb[:], in_=WdT_d[1])
            nc.sync.dma_start(out=mixT_sb[:], in_=mixT_v[:])
            nc.sync.dma_start(out=ones_sb[:], in_=ones_d[:])
            make_identity(nc, id2_sb[:])
            # ones columns of v' (Z accumulator rows); bf16 memset is valid
            nc.vector.memset(v_sb[:, :, 64:65], 1.0)
            nc.vector.memset(v_sb[:, :, 129:130], 1.0)

            def emit_qTin_dma(lg):
                lsl = slice(lg * LW, (lg + 1) * LW)
                qTin_sb = main.tile([128, DT, LW], F32R, tag="qin", name=f"qin{lg}")
                nc.sync.dma_start(out=qTin_sb[:], in_=qT_v[:, :, lsl])
                return qTin_sb

            def emit_qproj(lg, psum_pool, qTin_sb=None, ptag="mm"):
                if qTin_sb is None:
                    qTin_sb = emit_qTin_dma(lg)
                qTc_sb = main.tile(
                    [128, DT, LW], F32, tag="qtc", bufs=1, name=f"qtc{lg}"
                )
                for et in range(DT):
                    ps_q = psum_pool.tile(
                        [128, LW], F32, tag=ptag, name=f"psq{lg}_{et}"
                    )
                    for dt_ in range(DT):
                        nc.tensor.matmul(
                            ps_q[:],
                            WqT_sb[:, dt_, et * 128 : (et + 1) * 128],
                            qTin_sb[:, dt_, :],
                            start=(dt_ == 0),
                            stop=(dt_ == DT - 1),
                        )
                    nc.scalar.copy(qTc_sb[:, et, :], ps_q[:])
                mqs = {}
                for h in range(2):
                    mq_sb = main.tile(
                        [128, DT, LW], F32R, tag=f"mq{h}", bufs=1, name=f"mq{lg}_{h}"
                    )
                    for et in range(DT):
                        nc.vector.tensor_scalar_mul(
                            mq_sb[:, et, :],
                            qTc_sb[:, et, :],
                            mixT_sb[:, et, h : h + 1],
                        )
                    mqs[h] = mq_sb
                return mqs

            # ---------------- prep phase ----------------
            with (
                tc.tile_pool(name="prep", bufs=1) as prep,
                tc.tile_pool(name="ps_prep", bufs=2, space="PSUM") as ps_prep,
            ):
                WkT_sb = prep.tile([128, DT, D], F32R, tag="wk")
                WvT_sb = prep.tile([128, DT, 256], F32R, tag="wv")
                WcbT_sb = prep.tile([128, DT, 2], F32R, tag="wcb")
                cbT_sb = prep.tile([2, S], F32, tag="cbt")

                # lg0 query projection first: PE starts on 2MB of input
                # instead of idling until the 10MB prep inflow lands
                mqs = emit_qproj(0, ps_prep)

                nc.scalar.dma_start(out=WkT_sb[:], in_=WkT_v[:])
                nc.scalar.dma_start(out=WvT_sb[:], in_=WvT_v[:])
                nc.scalar.dma_start(out=WcbT_sb[:], in_=WcbT_v[:])

                # kT[e, s] = Wk @ keys.T and cb rows, chunked by s-group
                for sg in range(4):
                    ssl = slice(sg * 512, (sg + 1) * 512)
                    kin_sb = prep.tile(
                        [128, DT, 512], F32R, tag="pin", bufs=2, name=f"kin{sg}"
                    )
                    nc.scalar.dma_start(out=kin_sb[:], in_=kT_v[:, :, ssl])
                    for et in range(DT):
                        ps_k = ps_prep.tile([128, 512], F32, tag="mm")
                        for dt_ in range(DT):
                            nc.tensor.matmul(
                                ps_k[:],
                                WkT_sb[:, dt_, et * 128 : (et + 1) * 128],
                                kin_sb[:, dt_, :],
                                start=(dt_ == 0),
                                stop=(dt_ == DT - 1),
                            )
                        nc.vector.tensor_copy(kT_sb[:, et, ssl], ps_k[:])
                    ps_cb = ps_prep.tile([2, 512], F32, tag="cb")
                    for dt_ in range(DT):
                        nc.tensor.matmul(
                            ps_cb[:],
                            WcbT_sb[:, dt_, :],
                            kin_sb[:, dt_, :],
                            start=(dt_ == 0),
                            stop=(dt_ == DT - 1),
                        )
                    nc.scalar.mul(cbT_sb[:, ssl], ps_cb[:], INV_SQRT_HEAD)
                for st in range(ST):
                    ps_cbt = ps_prep.tile([128, 2], F32, tag="cbtr")
                    nc.tensor.transpose(
                        ps_cbt[:], cbT_sb[:, st * 128 : (st + 1) * 128], id2_sb[:]
                    )
                    nc.vector.tensor_copy(cb8_sb[:, st, :], ps_cbt[:])

                # v[s, v'] = values @ Wv.T (pair slice; ones col stays 1)
                for sg in range(4):
                    ssl = slice(sg * 512, (sg + 1) * 512)
                    vin_sb = prep.tile(
                        [128, DT, 512], F32R, tag="pin", bufs=2, name=f"vin{sg}"
                    )
                    nc.gpsimd.dma_start(out=vin_sb[:], in_=vT_v[:, :, ssl])
                    for sti in range(4):
                        st = sg * 4 + sti
                        ps_v = ps_prep.tile([128, 256], F32, tag="mm")
                        for dt_ in range(DT):
                            nc.tensor.matmul(
                                ps_v[:],
                                vin_sb[:, dt_, sti * 128 : (sti + 1) * 128],
                                WvT_sb[:, dt_, :],
                                start=(dt_ == 0),
                                stop=(dt_ == DT - 1),
                            )
                        nc.vector.tensor_copy(v_sb[:, st, 0:64], ps_v[:, 0:64])
                        nc.vector.tensor_copy(v_sb[:, st, 65:129], ps_v[:, 64:128])

            # ---------------- main loop (head-pipelined) ----------------
            # Head k's normalization/probs epilogue is emitted after head
            # k+1's score matmuls so the PE never waits on the reciprocal
            # chain; E tiles alternate between two bf16 tag sets.
            with (
                tc.tile_pool(name="ps_s", bufs=4, space="PSUM") as ps_s,
                tc.tile_pool(name="ps_ctx", bufs=2, space="PSUM") as ps_ctx,
                tc.tile_pool(name="ps_misc", bufs=2, space="PSUM") as ps_misc,
            ):
                ctxn = {}

                def emit_scores_ctx(lg, h, mq_sb):
                    par = (2 * lg + h) % 2
                    pc = ps_ctx.tile([65, LW], F32, tag="ctx", name=f"pc{lg}_{h}")
                    e_tiles = []
                    for st in range(ST):
                        ps_sc = ps_s.tile([128, LW], F32, tag="s", name=f"s{lg}{h}{st}")
                        for et in range(DT):
                            nc.tensor.matmul(
                                ps_sc[:],
                                kT_sb[:, et, st * 128 : (st + 1) * 128],
                                mq_sb[:, et, :],
                                start=(et == 0),
                                stop=(et == DT - 1),
                            )
                        e_sb = epool.tile(
                            [128, LW], BF16, tag=f"e{st}p{par}", name=f"e{lg}{h}{st}"
                        )
                        nc.scalar.activation(
                            e_sb[:],
                            ps_sc[:],
                            mybir.ActivationFunctionType.Exp,
                            bias=cb8_sb[:, st, h : h + 1],
                            scale=INV_SQRT_HEAD,
                        )
                        e_tiles.append(e_sb)
                        nc.tensor.matmul(
                            pc[:],
                            v_sb[:, st, 65 * h : 65 * h + 65],
                            e_sb[:],
                            start=(st == 0),
                            stop=(st == ST - 1),
                            skip_group_check=True,
                        )
                    return pc, e_tiles

                def emit_epilogue(lg, h, pc, e_tiles, tail=False):
                    lsl = slice(lg * LW, (lg + 1) * LW)
                    rz_sb = main.tile(
                        [65, LW], F32, tag=f"rz{h}", bufs=1, name=f"rz{lg}{h}"
                    )
                    nc.vector.reciprocal(rz_sb[64:65, :], pc[64:65, :])
                    rzr_sb = main.tile(
                        [65, LW], F32R, tag=f"rzr{h}", bufs=1, name=f"rzr{lg}{h}"
                    )
                    nc.vector.tensor_copy(rzr_sb[64:65, :], rz_sb[64:65, :])
                    ps_rb = ps_misc.tile([128, LW], F32, tag="mm", name=f"prb{lg}{h}")
                    nc.tensor.matmul(
                        ps_rb[:],
                        ones_sb[64:65, :],
                        rzr_sb[64:65, :],
                        start=True,
                        stop=True,
                    )
                    rbc_sb = main.tile(
                        [128, LW], F32, tag=f"rbc{h}", bufs=1, name=f"rbc{lg}{h}"
                    )
                    nc.vector.tensor_copy(rbc_sb[:], ps_rb[:])

                    cn_sb = main.tile(
                        [HD, LW], F32R, tag=f"cn{h}", bufs=1, name=f"cn{lg}{h}"
                    )
                    nc.vector.tensor_mul(cn_sb[:], pc[0:64, :], rbc_sb[0:64, :])
                    ctxn[(lg, h)] = cn_sb

                    # probs tiles: split between DVE and GpSimd so neither
                    # engine serializes the epilogue
                    for st in range(ST):
                        p_sb = stage.tile([128, LW], F32, tag="probs", name=f"p{lg}{h}{st}")
                        if tail:
                            ve = nc.vector if st % 4 != 3 else nc.gpsimd
                        else:
                            ve = nc.vector if st % 2 == 0 else nc.gpsimd
                        ve.tensor_mul(p_sb[:], e_tiles[st][:], rbc_sb[:])
                        nc.sync.dma_start(
                            out=probsT_d[h, st * 128 : (st + 1) * 128, lsl],
                            in_=p_sb[:],
                        )

                def emit_dense(lg):
                    lsl = slice(lg * LW, (lg + 1) * LW)
                    for ot in range(4):
                        ps_o = ps_misc.tile([128, LW], F32, tag="mm", name=f"po{lg}{ot}")
                        for h in range(2):
                            wd = WdT0_sb if h == 0 else WdT1_sb
                            nc.tensor.matmul(
                                ps_o[:],
                                wd[:, ot * 128 : (ot + 1) * 128],
                                ctxn[(lg, h)][:],
                                start=(h == 0),
                                stop=(h == 1),
                            )
                        o_sb = stage.tile([128, LW], F32, tag="out", bufs=3, name=f"o{lg}{ot}")
                        nc.scalar.copy(o_sb[:], ps_o[:])
                        nc.sync.dma_start(
                            out=outT_d[ot * 128 : (ot + 1) * 128, lsl], in_=o_sb[:]
                        )

                pending = None
                mqs_by_lg = {0: mqs}
                for lg in range(LG):
                    if lg > 0:
                        mqs_by_lg[lg] = emit_qproj(lg, ps_s, ptag="s")
                    for h in range(2):
                        pc, e_tiles = emit_scores_ctx(lg, h, mqs_by_lg[lg][h])
                        if pending is not None:
                            emit_epilogue(*pending)
                            if pending[1] == 1:
                                emit_dense(pending[0])
                        pending = (lg, h, pc, e_tiles)
                emit_epilogue(*pending, tail=True)
                emit_dense(LG - 1)

    _split_multi_waits(nc)
    return nc


_NC_CACHE = None


def _get_program():
    global _NC_CACHE
    if _NC_CACHE is None:
        _NC_CACHE = _build_program()
    return _NC_CACHE


def kernel(queries, keys, values, attn_mask, Wq, Wk, Wv, bv, Wcb, mixing, Wd, bd):
    queries = np.asarray(queries, np.float32)
    keys = np.asarray(keys, np.float32)
    values = np.asarray(values, np.float32)
    Wq = np.asarray(Wq, np.float32)
    Wk = np.asarray(Wk, np.float32)
    Wv = np.asarray(Wv, np.float32)
    bv = np.asarray(bv, np.float32)
    Wcb = np.asarray(Wcb, np.float32)
    mixing = np.asarray(mixing, np.float32)
    Wd = np.asarray(Wd, np.float32)
    bd = np.asarray(bd, np.float32)

    WqT = np.ascontiguousarray(Wq.T)
    WkT = np.ascontiguousarray(Wk.T)
    WvT_full = np.ascontiguousarray(Wv.T)          # [d, v]
    WcbT = np.ascontiguousarray(Wcb.T)             # [d, h]
    WdT_full = np.ascontiguousarray(Wd.T)          # [v, o]
    mixT = np.ascontiguousarray(mixing.T)          # [e, h]

    in_maps = []
    for core in range(8):
        b, p = divmod(core, PAIRS)
        vcols = np.zeros((D, 256), np.float32)
        vcols[:, 0:128] = WvT_full[:, 128 * p : 128 * p + 128]
        in_maps.append(
            {
                "qT": np.ascontiguousarray(queries[b].T),
                "kTin": np.ascontiguousarray(keys[b].T),
                "vTin": np.ascontiguousarray(values[b].T),
                "WqT": WqT,
                "WkT": WkT,
                "WvT": vcols,
                "WcbT": np.ascontiguousarray(WcbT[:, 2 * p : 2 * p + 2]),
                "mixT": np.ascontiguousarray(mixT[:, 2 * p : 2 * p + 2]),
                "WdT": np.ascontiguousarray(
                    WdT_full[128 * p : 128 * p + 128].reshape(2, HD, D)
                ),
                "ones": np.ones((128, 128), np.float32),
            }
        )

    nc = _get_program()
    trace = bool(int(os.environ.get("KERNEL_TRACE", "0")))
    trace_dir = os.environ.get("KERNEL_TRACE_DIR") or None
    res = run_bass_kernel_spmd(
        nc, in_maps, list(range(8)), trace=trace, tmpdir=trace_dir
    )
    if trace and res.exec_time_ns is not None:
        print(f"HW exec time: {res.exec_time_ns} ns")
        kernel.last_exec_time_ns = res.exec_time_ns

    out = np.empty((B, L, D), np.float32)
    probs = np.empty((B, H, L, S), np.float32)
    bias = bd + Wd @ bv  # bv folds into out because softmax rows sum to 1
    for b in range(B):
        acc = np.zeros((D, L), np.float32)
        for p in range(PAIRS):
            r = res.results[4 * b + p]
            acc += r["outT"]
            probs[b, 2 * p] = r["probsT"][0].T
            probs[b, 2 * p + 1] = r["probsT"][1].T
        out[b] = acc.T + bias
    return out, probs


kernel.last_exec_time_ns = None


# revision 34
# speedup vs baseline: 1.0284x; 1.0284x over previous
"""Trainium2 Bass kernel for the collaborative attention layer.

Shapes (hardcoded): B=2, L=S=2048, DIN=DK=DV=DOUT=512, H=8.
Sharding over 8 cores: core i handles batch b=i//4 and head pair
{2*(i%4), 2*(i%4)+1} (data-parallel on B, tensor-parallel on H).

Per-core device program (SPMD, identical program, different shards):
  kT   = Wk @ keys.T          [e, s]   (fp32r matmuls, contract d on partitions)
  cbT  = Wcb @ keys.T / 8     [2, s]  -> PE-transposed into per-partition layout
  v    = values @ Wv.T        [s, v]   plus an appended ones column per head
  per l-group of 512, per head:
    qT chunk = Wq @ queries.T [e, l]
    mqT      = qT * mixing_h  (per-partition scalar)
    scoresT  = kT.T-tiles @ mqT -> PSUM [s_tile, l]
    E        = exp(scoresT/8 + cb_h/8)   (ACT, bias per partition)
    ctxT_u/Z = [v_h | 1].T @ E  -> PSUM [65, l]  (row 64 = softmax denom Z)
    R        = 1/Z; Rbc = ones.T @ R (PE rank-1 broadcast)
    probsT   = E * Rbc  -> DMA out (transposed layout, host fixes with a view)
    ctxT_n   = ctxT_u * Rbc
  outT partial = Wd_pair.T-tiles @ [ctxT_n(h0); ctxT_n(h1)] -> DMA out

Host adds the per-batch partials, bd, and Wd @ bv (bias algebra: adding bv to v
adds (Wd @ bv) to out because softmax rows sum to 1), and transposes views.
"""

import os
import sys

sys.path.insert(0, "/opt/trn_rl_repo")

import ml_dtypes
import numpy as np

import concourse.bass as bass
import concourse.mybir as mybir
import concourse.tile as tile
from concourse.bass_utils import run_bass_kernel_spmd
from concourse.masks import make_identity
from concourse.vector_clock import ScopedClock

B, L, S, D, H = 2, 2048, 2048, 512, 8
HD = D // H          # 64 per-head value dim
PAIRS = 4            # head pairs per batch
F32 = mybir.dt.float32
F32R = mybir.dt.float32r
BF16 = mybir.dt.bfloat16
INV_SQRT_HEAD = 1.0 / 8.0  # 1/sqrt(DK/H) = 1/sqrt(64)

DT = D // 128        # 4 contraction tiles of 128
ST = S // 128        # 16 key/value tiles of 128
LG = 4               # l groups
LW = L // LG         # 512 moving width


def _split_multi_waits(nc: bass.Bass) -> None:
    """This walrus build rejects instructions carrying more than one sync
    wait (CoreV3 setupSyncWait). Rewrite any multi-wait instruction into
    single-wait EventSemaphore carriers (what wait_ge lowers to) followed
    by the original instruction with its last wait - identical semantics,
    since waits on the same engine queue AND sequentially."""
    uid = 0
    for f in nc.m.functions:
        for bb in f.blocks:
            insts = bb.instructions
            i = 0
            while i < len(insts):
                inst = insts[i]
                si = inst.sync_info
                if si is not None and si.on_wait and len(si.on_wait) > 1:
                    waits = list(si.on_wait)
                    si.on_wait = waits[-1:]
                    for w in waits[:-1]:
                        carrier = mybir.InstEventSemaphore(
                            name=f"waitsplit-{uid}", ins=[], outs=[]
                        )
                        uid += 1
                        carrier.engine = inst.engine
                        carrier.sync_info = mybir.SyncInfo(on_wait=[w], on_update=[])
                        insts.insert(i, carrier)
                        i += 1
                i += 1


class _SplitDrainTileContext(tile.TileContext):
    """Kept as a plain alias; multi-wait splitting happens in
    _split_multi_waits after the TileContext exits."""


def _r(ap):
    return ap.bitcast(F32R)


def _build_program() -> bass.Bass:
    nc = bass.Bass()

    # Matmul-fed inputs are declared float32r (same bits as f32; the BIR
    # verifier requires fp32r matmul operands to come from fp32r-typed
    # producers).
    qT_d = nc.declare_dram_parameter("qT", [D, L], F32R, isOutput=False)
    kT_d = nc.declare_dram_parameter("kTin", [D, S], F32R, isOutput=False)
    vT_d = nc.declare_dram_parameter("vTin", [D, S], F32R, isOutput=False)
    WqT_d = nc.declare_dram_parameter("WqT", [D, D], F32R, isOutput=False)
    WkT_d = nc.declare_dram_parameter("WkT", [D, D], F32R, isOutput=False)
    WvT_d = nc.declare_dram_parameter("WvT", [D, 256], F32R, isOutput=False)
    WcbT_d = nc.declare_dram_parameter("WcbT", [D, 2], F32R, isOutput=False)
    mixT_d = nc.declare_dram_parameter("mixT", [D, 2], F32, isOutput=False)
    WdT_d = nc.declare_dram_parameter("WdT", [2, HD, D], F32R, isOutput=False)
    ones_d = nc.declare_dram_parameter("ones", [128, 128], F32R, isOutput=False)

    probsT_d = nc.declare_dram_parameter("probsT", [2, S, L], F32, isOutput=True)
    outT_d = nc.declare_dram_parameter("outT", [D, L], F32, isOutput=True)

    # d-major tiled views: row d = t*128 + p  ->  [p, t, n]
    qT_v = qT_d.rearrange("(t p) n -> p t n", p=128)
    kT_v = kT_d.rearrange("(t p) n -> p t n", p=128)
    vT_v = vT_d.rearrange("(t p) n -> p t n", p=128)
    WqT_v = WqT_d.rearrange("(t p) n -> p t n", p=128)
    WkT_v = WkT_d.rearrange("(t p) n -> p t n", p=128)
    WvT_v = WvT_d.rearrange("(t p) n -> p t n", p=128)
    WcbT_v = WcbT_d.rearrange("(t p) n -> p t n", p=128)
    mixT_v = mixT_d.rearrange("(t p) n -> p t n", p=128)

    with tile.TileContext(nc) as tc:
        with (
            tc.tile_pool(name="persist", bufs=1) as persist,
            tc.tile_pool(name="main", bufs=2) as main,
            tc.tile_pool(name="epool", bufs=1) as epool,
            tc.tile_pool(name="stage", bufs=4) as stage,
        ):
            # ---------------- persistent tiles ----------------
            kT_sb = persist.tile([128, DT, S], F32R)      # 32KB/part
            v_sb = persist.tile([128, ST, 132], BF16)     # [s, v'] per head pair
            WqT_sb = persist.tile([128, DT, D], F32R)
            WdT0_sb = persist.tile([HD, D], F32R)
            WdT1_sb = persist.tile([HD, D], F32R)
            mixT_sb = persist.tile([128, DT, 2], F32)
            cb8_sb = persist.tile([128, ST, 2], F32)      # cb/8, s on partitions
            ones_sb = persist.tile([128, 128], F32R)
            id2_sb = persist.tile([2, 2], F32)

            nc.sync.dma_start(out=WqT_sb[:], in_=WqT_v[:])
            nc.sync.dma_start(out=WdT0_sb[:], in_=WdT_d[0])
            nc.sync.dma_start(out=WdT1_sb[:], in_=WdT_d[1])
            nc.sync.dma_start(out=mixT_sb[:], in_=mixT_v[:])
            nc.sync.dma_start(out=ones_sb[:], in_=ones_d[:])
            make_identity(nc, id2_sb[:])
            # ones columns of v' (Z accumulator rows); bf16 memset is valid
            nc.vector.memset(v_sb[:, :, 64:65], 1.0)
            nc.vector.memset(v_sb[:, :, 129:130], 1.0)

            def emit_qTin_dma(lg):
                lsl = slice(lg * LW, (lg + 1) * LW)
                qTin_sb = main.tile([128, DT, LW], F32R, tag="qin", name=f"qin{lg}")
                nc.sync.dma_start(out=qTin_sb[:], in_=qT_v[:, :, lsl])
                return qTin_sb

            def emit_qproj(lg, psum_pool, qTin_sb=None, ptag="mm"):
                if qTin_sb is None:
                    qTin_sb = emit_qTin_dma(lg)
                qTc_sb = main.tile(
                    [128, DT, LW], F32, tag="qtc", bufs=1, name=f"qtc{lg}"
                )
                for et in range(DT):
                    ps_q = psum_pool.tile(
                        [128, LW], F32, tag=ptag, name=f"psq{lg}_{et}"
                    )
                    for dt_ in range(DT):
                        nc.tensor.matmul(
                            ps_q[:],
                            WqT_sb[:, dt_, et * 128 : (et + 1) * 128],
                            qTin_sb[:, dt_, :],
                            start=(dt_ == 0),
                            stop=(dt_ == DT - 1),
                        )
                    nc.scalar.copy(qTc_sb[:, et, :], ps_q[:])
                mqs = {}
                for h in range(2):
                    mq_sb = main.tile(
                        [128, DT, LW], F32R, tag=f"mq{h}", bufs=1, name=f"mq{lg}_{h}"
                    )
                    for et in range(DT):
                        nc.vector.tensor_scalar_mul(
                            mq_sb[:, et, :],
                            qTc_sb[:, et, :],
                            mixT_sb[:, et, h : h + 1],
                        )
                    mqs[h] = mq_sb
                return mqs

            # ---------------- prep phase ----------------
            with (
                tc.tile_pool(name="prep", bufs=1) as prep,
                tc.tile_pool(name="ps_prep", bufs=2, space="PSUM") as ps_prep,
            ):
                WkT_sb = prep.tile([128, DT, D], F32R, tag="wk")
                WvT_sb = prep.tile([128, DT, 256], F32R, tag="wv")
                WcbT_sb = prep.tile([128, DT, 2], F32R, tag="wcb")
                cbT_sb = prep.tile([2, S], F32, tag="cbt")

                # lg0 query projection first: PE starts on 2MB of input
                # instead of idling until the 10MB prep inflow lands
                mqs = emit_qproj(0, ps_prep)

                nc.sync.dma_start(out=WkT_sb[:], in_=WkT_v[:])
                nc.sync.dma_start(out=WvT_sb[:], in_=WvT_v[:])
                nc.sync.dma_start(out=WcbT_sb[:], in_=WcbT_v[:])

                # kT[e, s] = Wk @ keys.T and cb rows, chunked by s-group
                for sg in range(4):
                    ssl = slice(sg * 512, (sg + 1) * 512)
                    kin_sb = prep.tile(
                        [128, DT, 512], F32R, tag="kin", bufs=2, name=f"kin{sg}"
                    )
                    nc.sync.dma_start(out=kin_sb[:], in_=kT_v[:, :, ssl])
                    for et in range(DT):
                        ps_k = ps_prep.tile([128, 512], F32, tag="mm")
                        for dt_ in range(DT):
                            nc.tensor.matmul(
                                ps_k[:],
                                WkT_sb[:, dt_, et * 128 : (et + 1) * 128],
                                kin_sb[:, dt_, :],
                                start=(dt_ == 0),
                                stop=(dt_ == DT - 1),
                            )
                        nc.vector.tensor_copy(kT_sb[:, et, ssl], ps_k[:])
                    ps_cb = ps_prep.tile([2, 512], F32, tag="cb")
                    for dt_ in range(DT):
                        nc.tensor.matmul(
                            ps_cb[:],
                            WcbT_sb[:, dt_, :],
                            kin_sb[:, dt_, :],
                            start=(dt_ == 0),
                            stop=(dt_ == DT - 1),
                        )
                    nc.scalar.mul(cbT_sb[:, ssl], ps_cb[:], INV_SQRT_HEAD)
                for st in range(ST):
                    ps_cbt = ps_prep.tile([128, 2], F32, tag="cbtr")
                    nc.tensor.transpose(
                        ps_cbt[:], cbT_sb[:, st * 128 : (st + 1) * 128], id2_sb[:]
                    )
                    nc.vector.tensor_copy(cb8_sb[:, st, :], ps_cbt[:])

                # v[s, v'] = values @ Wv.T (pair slice; ones col stays 1)
                for sg in range(4):
                    ssl = slice(sg * 512, (sg + 1) * 512)
                    vin_sb = prep.tile(
                        [128, DT, 512], F32R, tag="vin", bufs=2, name=f"vin{sg}"
                    )
                    nc.sync.dma_start(out=vin_sb[:], in_=vT_v[:, :, ssl])
                    for sti in range(4):
                        st = sg * 4 + sti
                        ps_v = ps_prep.tile([128, 256], F32, tag="mm")
                        for dt_ in range(DT):
                            nc.tensor.matmul(
                                ps_v[:],
                                vin_sb[:, dt_, sti * 128 : (sti + 1) * 128],
                                WvT_sb[:, dt_, :],
                                start=(dt_ == 0),
                                stop=(dt_ == DT - 1),
                            )
                        nc.vector.tensor_copy(v_sb[:, st, 0:64], ps_v[:, 0:64])
                        nc.vector.tensor_copy(v_sb[:, st, 65:129], ps_v[:, 64:128])

            # ---------------- main loop (head-pipelined) ----------------
            # Head k's normalization/probs epilogue is emitted after head
            # k+1's score matmuls so the PE never waits on the reciprocal
            # chain; E tiles alternate between two bf16 tag sets.
            with (
                tc.tile_pool(name="ps_s", bufs=4, space="PSUM") as ps_s,
                tc.tile_pool(name="ps_ctx", bufs=2, space="PSUM") as ps_ctx,
                tc.tile_pool(name="ps_misc", bufs=2, space="PSUM") as ps_misc,
            ):
                ctxn = {}

                def emit_scores_ctx(lg, h, mq_sb):
                    par = (2 * lg + h) % 2
                    pc = ps_ctx.tile([65, LW], F32, tag="ctx", name=f"pc{lg}_{h}")
                    e_tiles = []
                    for st in range(ST):
                        ps_sc = ps_s.tile([128, LW], F32, tag="s", name=f"s{lg}{h}{st}")
                        for et in range(DT):
                            nc.tensor.matmul(
                                ps_sc[:],
                                kT_sb[:, et, st * 128 : (st + 1) * 128],
                                mq_sb[:, et, :],
                                start=(et == 0),
                                stop=(et == DT - 1),
                            )
                        e_sb = epool.tile(
                            [128, LW], BF16, tag=f"e{st}p{par}", name=f"e{lg}{h}{st}"
                        )
                        nc.scalar.activation(
                            e_sb[:],
                            ps_sc[:],
                            mybir.ActivationFunctionType.Exp,
                            bias=cb8_sb[:, st, h : h + 1],
                            scale=INV_SQRT_HEAD,
                        )
                        e_tiles.append(e_sb)
                        nc.tensor.matmul(
                            pc[:],
                            v_sb[:, st, 65 * h : 65 * h + 65],
                            e_sb[:],
                            start=(st == 0),
                            stop=(st == ST - 1),
                            skip_group_check=True,
                        )
                    return pc, e_tiles

                def emit_epilogue(lg, h, pc, e_tiles, tail=False):
                    lsl = slice(lg * LW, (lg + 1) * LW)
                    rz_sb = main.tile(
                        [65, LW], F32, tag=f"rz{h}", bufs=1, name=f"rz{lg}{h}"
                    )
                    nc.vector.reciprocal(rz_sb[64:65, :], pc[64:65, :])
                    rzr_sb = main.tile(
                        [65, LW], F32R, tag=f"rzr{h}", bufs=1, name=f"rzr{lg}{h}"
                    )
                    nc.vector.tensor_copy(rzr_sb[64:65, :], rz_sb[64:65, :])
                    ps_rb = ps_misc.tile([128, LW], F32, tag="mm", name=f"prb{lg}{h}")
                    nc.tensor.matmul(
                        ps_rb[:],
                        ones_sb[64:65, :],
                        rzr_sb[64:65, :],
                        start=True,
                        stop=True,
                    )
                    rbc_sb = main.tile(
                        [128, LW], F32, tag=f"rbc{h}", bufs=1, name=f"rbc{lg}{h}"
                    )
                    nc.vector.tensor_copy(rbc_sb[:], ps_rb[:])

                    cn_sb = main.tile(
                        [HD, LW], F32R, tag=f"cn{h}", bufs=1, name=f"cn{lg}{h}"
                    )
                    nc.vector.tensor_mul(cn_sb[:], pc[0:64, :], rbc_sb[0:64, :])
                    ctxn[(lg, h)] = cn_sb

                    # probs tiles: split between DVE and GpSimd so neither
                    # engine serializes the epilogue
                    for st in range(ST):
                        p_sb = stage.tile([128, LW], F32, tag="probs", name=f"p{lg}{h}{st}")
                        if tail:
                            ve = nc.vector if st % 4 != 3 else nc.gpsimd
                        else:
                            ve = nc.vector if st % 2 == 0 else nc.gpsimd
                        ve.tensor_mul(p_sb[:], e_tiles[st][:], rbc_sb[:])
                        nc.sync.dma_start(
                            out=probsT_d[h, st * 128 : (st + 1) * 128, lsl],
                            in_=p_sb[:],
                        )

                def emit_dense(lg):
                    lsl = slice(lg * LW, (lg + 1) * LW)
                    for ot in range(4):
                        ps_o = ps_misc.tile([128, LW], F32, tag="mm", name=f"po{lg}{ot}")
                        for h in range(2):
                            wd = WdT0_sb if h == 0 else WdT1_sb
                            nc.tensor.matmul(
                                ps_o[:],
                                wd[:, ot * 128 : (ot + 1) * 128],
                                ctxn[(lg, h)][:],
                                start=(h == 0),
                                stop=(h == 1),
                            )
                        o_sb = stage.tile([128, LW], F32, tag="out", bufs=3, name=f"o{lg}{ot}")
                        nc.scalar.copy(o_sb[:], ps_o[:])
                        nc.sync.dma_start(
                            out=outT_d[ot * 128 : (ot + 1) * 128, lsl], in_=o_sb[:]
                        )

                pending = None
                mqs_by_lg = {0: mqs}
                for lg in range(LG):
                    if lg > 0:
                        mqs_by_lg[lg] = emit_qproj(lg, ps_misc)
                    for h in range(2):
                        pc, e_tiles = emit_scores_ctx(lg, h, mqs_by_lg[lg][h])
                        if pending is not None:
                            emit_epilogue(*pending)
                            if pending[1] == 1:
                                emit_dense(pending[0])
                        pending = (lg, h, pc, e_tiles)
                emit_epilogue(*pending, tail=True)
                emit_dense(LG - 1)

    _split_multi_waits(nc)
    return nc


_NC_CACHE = None


def _get_program():
    global _NC_CACHE
    if _NC_CACHE is None:
        _NC_CACHE = _build_program()
    return _NC_CACHE


def kernel(queries, keys, values, attn_mask, Wq, Wk, Wv, bv, Wcb, mixing, Wd, bd):
    queries = np.asarray(queries, np.float32)
    keys = np.asarray(keys, np.float32)
    values = np.asarray(values, np.float32)
    Wq = np.asarray(Wq, np.float32)
    Wk = np.asarray(Wk, np.float32)
    Wv = np.asarray(Wv, np.float32)
    bv = np.asarray(bv, np.float32)
    Wcb = np.asarray(Wcb, np.float32)
    mixing = np.asarray(mixing, np.float32)
    Wd = np.asarray(Wd, np.float32)
    bd = np.asarray(bd, np.float32)

    WqT = np.ascontiguousarray(Wq.T)
    WkT = np.ascontiguousarray(Wk.T)
    WvT_full = np.ascontiguousarray(Wv.T)          # [d, v]
    WcbT = np.ascontiguousarray(Wcb.T)             # [d, h]
    WdT_full = np.ascontiguousarray(Wd.T)          # [v, o]
    mixT = np.ascontiguousarray(mixing.T)          # [e, h]

    in_maps = []
    for core in range(8):
        b, p = divmod(core, PAIRS)
        vcols = np.zeros((D, 256), np.float32)
        vcols[:, 0:128] = WvT_full[:, 128 * p : 128 * p + 128]
        in_maps.append(
            {
                "qT": np.ascontiguousarray(queries[b].T),
                "kTin": np.ascontiguousarray(keys[b].T),
                "vTin": np.ascontiguousarray(values[b].T),
                "WqT": WqT,
                "WkT": WkT,
                "WvT": vcols,
                "WcbT": np.ascontiguousarray(WcbT[:, 2 * p : 2 * p + 2]),
                "mixT": np.ascontiguousarray(mixT[:, 2 * p : 2 * p + 2]),
                "WdT": np.ascontiguousarray(
                    WdT_full[128 * p : 128 * p + 128].reshape(2, HD, D)
                ),
                "ones": np.ones((128, 128), np.float32),
            }
        )

    nc = _get_program()
    trace = bool(int(os.environ.get("KERNEL_TRACE", "0")))
    trace_dir = os.environ.get("KERNEL_TRACE_DIR") or None
    res = run_bass_kernel_spmd(
        nc, in_maps, list(range(8)), trace=trace, tmpdir=trace_dir
    )
    if trace and res.exec_time_ns is not None:
        print(f"HW exec time: {res.exec_time_ns} ns")
        kernel.last_exec_time_ns = res.exec_time_ns

    out = np.empty((B, L, D), np.float32)
    probs = np.empty((B, H, L, S), np.float32)
    bias = bd + Wd @ bv  # bv folds into out because softmax rows sum to 1
    for b in range(B):
        acc = np.zeros((D, L), np.float32)
        for p in range(PAIRS):
            r = res.results[4 * b + p]
            acc += r["outT"]
            probs[b, 2 * p] = r["probsT"][0].T
            probs[b, 2 * p + 1] = r["probsT"][1].T
        out[b] = acc.T + bias
    return out, probs


kernel.last_exec_time_ns = None


# revision 35
# speedup vs baseline: 1.0627x; 1.0334x over previous
"""Trainium2 Bass kernel for the collaborative attention layer.

Shapes (hardcoded): B=2, L=S=2048, DIN=DK=DV=DOUT=512, H=8.
Sharding over 8 cores: core i handles batch b=i//4 and head pair
{2*(i%4), 2*(i%4)+1} (data-parallel on B, tensor-parallel on H).

Per-core device program (SPMD, identical program, different shards):
  kT   = Wk @ keys.T          [e, s]   (fp32r matmuls, contract d on partitions)
  cbT  = Wcb @ keys.T / 8     [2, s]  -> PE-transposed into per-partition layout
  v    = values @ Wv.T        [s, v]   plus an appended ones column per head
  per l-group of 512, per head:
    qT chunk = Wq @ queries.T [e, l]
    mqT      = qT * mixing_h  (per-partition scalar)
    scoresT  = kT.T-tiles @ mqT -> PSUM [s_tile, l]
    E        = exp(scoresT/8 + cb_h/8)   (ACT, bias per partition)
    ctxT_u/Z = [v_h | 1].T @ E  -> PSUM [65, l]  (row 64 = softmax denom Z)
    R        = 1/Z; Rbc = ones.T @ R (PE rank-1 broadcast)
    probsT   = E * Rbc  -> DMA out (transposed layout, host fixes with a view)
    ctxT_n   = ctxT_u * Rbc
  outT partial = Wd_pair.T-tiles @ [ctxT_n(h0); ctxT_n(h1)] -> DMA out

Host adds the per-batch partials, bd, and Wd @ bv (bias algebra: adding bv to v
adds (Wd @ bv) to out because softmax rows sum to 1), and transposes views.
"""

import os
import sys

sys.path.insert(0, "/opt/trn_rl_repo")

import ml_dtypes
import numpy as np

import concourse.bass as bass
import concourse.mybir as mybir
import concourse.tile as tile
from concourse.bass_utils import run_bass_kernel_spmd
from concourse.masks import make_identity
from concourse.vector_clock import ScopedClock

B, L, S, D, H = 2, 2048, 2048, 512, 8
HD = D // H          # 64 per-head value dim
PAIRS = 4            # head pairs per batch
F32 = mybir.dt.float32
F32R = mybir.dt.float32r
BF16 = mybir.dt.bfloat16
INV_SQRT_HEAD = 1.0 / 8.0  # 1/sqrt(DK/H) = 1/sqrt(64)

DT = D // 128        # 4 contraction tiles of 128
ST = S // 128        # 16 key/value tiles of 128
LG = 4               # l groups
LW = L // LG         # 512 moving width


def _split_multi_waits(nc: bass.Bass) -> None:
    """This walrus build rejects instructions carrying more than one sync
    wait (CoreV3 setupSyncWait). Rewrite any multi-wait instruction into
    single-wait EventSemaphore carriers (what wait_ge lowers to) followed
    by the original instruction with its last wait - identical semantics,
    since waits on the same engine queue AND sequentially."""
    uid = 0
    for f in nc.m.functions:
        for bb in f.blocks:
            insts = bb.instructions
            i = 0
            while i < len(insts):
                inst = insts[i]
                si = inst.sync_info
                if si is not None and si.on_wait and len(si.on_wait) > 1:
                    waits = list(si.on_wait)
                    si.on_wait = waits[-1:]
                    for w in waits[:-1]:
                        carrier = mybir.InstEventSemaphore(
                            name=f"waitsplit-{uid}", ins=[], outs=[]
                        )
                        uid += 1
                        carrier.engine = inst.engine
                        carrier.sync_info = mybir.SyncInfo(on_wait=[w], on_update=[])
                        insts.insert(i, carrier)
                        i += 1
                i += 1


class _SplitDrainTileContext(tile.TileContext):
    """Kept as a plain alias; multi-wait splitting happens in
    _split_multi_waits after the TileContext exits."""


def _r(ap):
    return ap.bitcast(F32R)


def _build_program() -> bass.Bass:
    nc = bass.Bass()

    # Matmul-fed inputs are declared float32r (same bits as f32; the BIR
    # verifier requires fp32r matmul operands to come from fp32r-typed
    # producers).
    qT_d = nc.declare_dram_parameter("qT", [D, L], F32R, isOutput=False)
    kT_d = nc.declare_dram_parameter("kTin", [D, S], F32R, isOutput=False)
    vT_d = nc.declare_dram_parameter("vTin", [D, S], F32R, isOutput=False)
    WqT_d = nc.declare_dram_parameter("WqT", [D, D], F32R, isOutput=False)
    WkT_d = nc.declare_dram_parameter("WkT", [D, D], F32R, isOutput=False)
    WvT_d = nc.declare_dram_parameter("WvT", [D, 256], F32R, isOutput=False)
    WcbT_d = nc.declare_dram_parameter("WcbT", [D, 2], F32R, isOutput=False)
    mixT_d = nc.declare_dram_parameter("mixT", [D, 2], F32, isOutput=False)
    WdT_d = nc.declare_dram_parameter("WdT", [2, HD, D], F32R, isOutput=False)
    ones_d = nc.declare_dram_parameter("ones", [128, 128], F32R, isOutput=False)

    probsT_d = nc.declare_dram_parameter("probsT", [2, S, L], F32, isOutput=True)
    outT_d = nc.declare_dram_parameter("outT", [D, L], F32, isOutput=True)

    # d-major tiled views: row d = t*128 + p  ->  [p, t, n]
    qT_v = qT_d.rearrange("(t p) n -> p t n", p=128)
    kT_v = kT_d.rearrange("(t p) n -> p t n", p=128)
    vT_v = vT_d.rearrange("(t p) n -> p t n", p=128)
    WqT_v = WqT_d.rearrange("(t p) n -> p t n", p=128)
    WkT_v = WkT_d.rearrange("(t p) n -> p t n", p=128)
    WvT_v = WvT_d.rearrange("(t p) n -> p t n", p=128)
    WcbT_v = WcbT_d.rearrange("(t p) n -> p t n", p=128)
    mixT_v = mixT_d.rearrange("(t p) n -> p t n", p=128)

    with tile.TileContext(nc) as tc:
        with (
            tc.tile_pool(name="persist", bufs=1) as persist,
            tc.tile_pool(name="main", bufs=2) as main,
            tc.tile_pool(name="epool", bufs=1) as epool,
            tc.tile_pool(name="stage", bufs=4) as stage,
        ):
            # ---------------- persistent tiles ----------------
            kT_sb = persist.tile([128, DT, S], F32R)      # 32KB/part
            v_sb = persist.tile([128, ST, 132], BF16)     # [s, v'] per head pair
            WqT_sb = persist.tile([128, DT, D], F32R)
            WdT0_sb = persist.tile([HD, D], F32R)
            WdT1_sb = persist.tile([HD, D], F32R)
            mixT_sb = persist.tile([128, DT, 2], F32)
            cb8_sb = persist.tile([128, ST, 2], F32)      # cb/8, s on partitions
            ones_sb = persist.tile([128, 128], F32R)
            id2_sb = persist.tile([2, 2], F32)

            nc.sync.dma_start(out=WqT_sb[:], in_=WqT_v[:])
            nc.sync.dma_start(out=WdT0_sb[:], in_=WdT_d[0])
            nc.sync.dma_start(out=WdT1_sb[:], in_=WdT_d[1])
            nc.sync.dma_start(out=mixT_sb[:], in_=mixT_v[:])
            nc.sync.dma_start(out=ones_sb[:], in_=ones_d[:])
            make_identity(nc, id2_sb[:])
            # ones columns of v' (Z accumulator rows); bf16 memset is valid
            nc.vector.memset(v_sb[:, :, 64:65], 1.0)
            nc.vector.memset(v_sb[:, :, 129:130], 1.0)

            def emit_qTin_dma(lg):
                lsl = slice(lg * LW, (lg + 1) * LW)
                qTin_sb = main.tile([128, DT, LW], F32R, tag="qin", name=f"qin{lg}")
                nc.sync.dma_start(out=qTin_sb[:], in_=qT_v[:, :, lsl])
                return qTin_sb

            def emit_qproj(lg, psum_pool, qTin_sb=None, ptag="mm"):
                if qTin_sb is None:
                    qTin_sb = emit_qTin_dma(lg)
                qTc_sb = main.tile(
                    [128, DT, LW], F32, tag="qtc", bufs=1, name=f"qtc{lg}"
                )
                for et in range(DT):
                    ps_q = psum_pool.tile(
                        [128, LW], F32, tag=ptag, name=f"psq{lg}_{et}"
                    )
                    for dt_ in range(DT):
                        nc.tensor.matmul(
                            ps_q[:],
                            WqT_sb[:, dt_, et * 128 : (et + 1) * 128],
                            qTin_sb[:, dt_, :],
                            start=(dt_ == 0),
                            stop=(dt_ == DT - 1),
                        )
                    nc.scalar.copy(qTc_sb[:, et, :], ps_q[:])
                mqs = {}
                for h in range(2):
                    mq_sb = main.tile(
                        [128, DT, LW], F32R, tag=f"mq{h}", bufs=1, name=f"mq{lg}_{h}"
                    )
                    for et in range(DT):
                        nc.vector.tensor_scalar_mul(
                            mq_sb[:, et, :],
                            qTc_sb[:, et, :],
                            mixT_sb[:, et, h : h + 1],
                        )
                    mqs[h] = mq_sb
                return mqs

            # ---------------- prep phase ----------------
            with (
                tc.tile_pool(name="prep", bufs=1) as prep,
                tc.tile_pool(name="ps_prep", bufs=2, space="PSUM") as ps_prep,
            ):
                WkT_sb = prep.tile([128, DT, D], F32R, tag="wk")
                WvT_sb = prep.tile([128, DT, 256], F32R, tag="wv")
                WcbT_sb = prep.tile([128, DT, 2], F32R, tag="wcb")
                cbT_sb = prep.tile([2, S], F32, tag="cbt")

                # lg0 query projection first: PE starts on 2MB of input
                # instead of idling until the 10MB prep inflow lands
                mqs = emit_qproj(0, ps_prep)

                nc.sync.dma_start(out=WkT_sb[:], in_=WkT_v[:])
                nc.sync.dma_start(out=WvT_sb[:], in_=WvT_v[:])
                nc.sync.dma_start(out=WcbT_sb[:], in_=WcbT_v[:])

                # kT[e, s] = Wk @ keys.T and cb rows, chunked by s-group
                for sg in range(4):
                    ssl = slice(sg * 512, (sg + 1) * 512)
                    kin_sb = prep.tile(
                        [128, DT, 512], F32R, tag="kin", bufs=2, name=f"kin{sg}"
                    )
                    nc.sync.dma_start(out=kin_sb[:], in_=kT_v[:, :, ssl])
                    for et in range(DT):
                        ps_k = ps_prep.tile([128, 512], F32, tag="mm")
                        for dt_ in range(DT):
                            nc.tensor.matmul(
                                ps_k[:],
                                WkT_sb[:, dt_, et * 128 : (et + 1) * 128],
                                kin_sb[:, dt_, :],
                                start=(dt_ == 0),
                                stop=(dt_ == DT - 1),
                            )
                        nc.vector.tensor_copy(kT_sb[:, et, ssl], ps_k[:])
                    ps_cb = ps_prep.tile([2, 512], F32, tag="cb")
                    for dt_ in range(DT):
                        nc.tensor.matmul(
                            ps_cb[:],
                            WcbT_sb[:, dt_, :],
                            kin_sb[:, dt_, :],
                            start=(dt_ == 0),
                            stop=(dt_ == DT - 1),
                        )
                    nc.scalar.mul(cbT_sb[:, ssl], ps_cb[:], INV_SQRT_HEAD)
                for st in range(ST):
                    ps_cbt = ps_prep.tile([128, 2], F32, tag="cbtr")
                    nc.tensor.transpose(
                        ps_cbt[:], cbT_sb[:, st * 128 : (st + 1) * 128], id2_sb[:]
                    )
                    nc.vector.tensor_copy(cb8_sb[:, st, :], ps_cbt[:])

                # v[s, v'] = values @ Wv.T (pair slice; ones col stays 1)
                for sg in range(4):
                    ssl = slice(sg * 512, (sg + 1) * 512)
                    vin_sb = prep.tile(
                        [128, DT, 512], F32R, tag="vin", bufs=2, name=f"vin{sg}"
                    )
                    nc.sync.dma_start(out=vin_sb[:], in_=vT_v[:, :, ssl])
                    for sti in range(4):
                        st = sg * 4 + sti
                        ps_v = ps_prep.tile([128, 256], F32, tag="mm")
                        for dt_ in range(DT):
                            nc.tensor.matmul(
                                ps_v[:],
                                vin_sb[:, dt_, sti * 128 : (sti + 1) * 128],
                                WvT_sb[:, dt_, :],
                                start=(dt_ == 0),
                                stop=(dt_ == DT - 1),
                            )
                        nc.vector.tensor_copy(v_sb[:, st, 0:64], ps_v[:, 0:64])
                        nc.vector.tensor_copy(v_sb[:, st, 65:129], ps_v[:, 64:128])

            # ---------------- main loop (head-pipelined) ----------------
            # Head k's normalization/probs epilogue is emitted after head
            # k+1's score matmuls so the PE never waits on the reciprocal
            # chain; E tiles alternate between two bf16 tag sets.
            with (
                tc.tile_pool(name="ps_s", bufs=4, space="PSUM") as ps_s,
                tc.tile_pool(name="ps_ctx", bufs=2, space="PSUM") as ps_ctx,
                tc.tile_pool(name="ps_misc", bufs=2, space="PSUM") as ps_misc,
            ):
                ctxn = {}

                def emit_scores_ctx(lg, h, mq_sb):
                    par = (2 * lg + h) % 2
                    pc = ps_ctx.tile([65, LW], F32, tag="ctx", name=f"pc{lg}_{h}")
                    e_tiles = []
                    for st in range(ST):
                        ps_sc = ps_s.tile([128, LW], F32, tag="s", name=f"s{lg}{h}{st}")
                        for et in range(DT):
                            nc.tensor.matmul(
                                ps_sc[:],
                                kT_sb[:, et, st * 128 : (st + 1) * 128],
                                mq_sb[:, et, :],
                                start=(et == 0),
                                stop=(et == DT - 1),
                            )
                        e_sb = epool.tile(
                            [128, LW], BF16, tag=f"e{st}p{par}", name=f"e{lg}{h}{st}"
                        )
                        nc.scalar.activation(
                            e_sb[:],
                            ps_sc[:],
                            mybir.ActivationFunctionType.Exp,
                            bias=cb8_sb[:, st, h : h + 1],
                            scale=INV_SQRT_HEAD,
                        )
                        e_tiles.append(e_sb)
                        nc.tensor.matmul(
                            pc[:],
                            v_sb[:, st, 65 * h : 65 * h + 65],
                            e_sb[:],
                            start=(st == 0),
                            stop=(st == ST - 1),
                            skip_group_check=True,
                        )
                    return pc, e_tiles

                def emit_epilogue(lg, h, pc, e_tiles, tail=False):
                    lsl = slice(lg * LW, (lg + 1) * LW)
                    rz_sb = main.tile(
                        [65, LW], F32, tag=f"rz{h}", bufs=1, name=f"rz{lg}{h}"
                    )
                    nc.vector.reciprocal(rz_sb[64:65, :], pc[64:65, :])
                    rzr_sb = main.tile(
                        [65, LW], F32R, tag=f"rzr{h}", bufs=1, name=f"rzr{lg}{h}"
                    )
                    nc.vector.tensor_copy(rzr_sb[64:65, :], rz_sb[64:65, :])
                    ps_rb = ps_misc.tile([128, LW], F32, tag="mm", name=f"prb{lg}{h}")
                    nc.tensor.matmul(
                        ps_rb[:],
                        ones_sb[64:65, :],
                        rzr_sb[64:65, :],
                        start=True,
                        stop=True,
                    )
                    rbc_sb = main.tile(
                        [128, LW], F32, tag=f"rbc{h}", bufs=1, name=f"rbc{lg}{h}"
                    )
                    nc.vector.tensor_copy(rbc_sb[:], ps_rb[:])

                    cn_sb = main.tile(
                        [HD, LW], F32R, tag=f"cn{h}", bufs=1, name=f"cn{lg}{h}"
                    )
                    nc.vector.tensor_mul(cn_sb[:], pc[0:64, :], rbc_sb[0:64, :])
                    ctxn[(lg, h)] = cn_sb

                    # probs tiles: split between DVE and GpSimd so neither
                    # engine serializes the epilogue
                    for st in range(ST):
                        p_sb = stage.tile([128, LW], F32, tag="probs", name=f"p{lg}{h}{st}")
                        if tail:
                            ve = nc.vector if st % 4 != 3 else nc.gpsimd
                        else:
                            ve = nc.vector if st % 2 == 0 else nc.gpsimd
                        ve.tensor_mul(p_sb[:], e_tiles[st][:], rbc_sb[:])
                        nc.sync.dma_start(
                            out=probsT_d[h, st * 128 : (st + 1) * 128, lsl],
                            in_=p_sb[:],
                        )

                def emit_dense(lg):
                    lsl = slice(lg * LW, (lg + 1) * LW)
                    for ot in range(4):
                        ps_o = ps_misc.tile([128, LW], F32, tag="mm", name=f"po{lg}{ot}")
                        for h in range(2):
                            wd = WdT0_sb if h == 0 else WdT1_sb
                            nc.tensor.matmul(
                                ps_o[:],
                                wd[:, ot * 128 : (ot + 1) * 128],
                                ctxn[(lg, h)][:],
                                start=(h == 0),
                                stop=(h == 1),
                            )
                        o_sb = stage.tile([128, LW], F32, tag="out", bufs=4, name=f"o{lg}{ot}")
                        nc.vector.tensor_copy(o_sb[:], ps_o[:])
                        nc.sync.dma_start(
                            out=outT_d[ot * 128 : (ot + 1) * 128, lsl], in_=o_sb[:]
                        )

                pending = None
                mqs_by_lg = {0: mqs}
                for lg in range(LG):
                    if lg > 0:
                        mqs_by_lg[lg] = emit_qproj(lg, ps_misc)
                    for h in range(2):
                        pc, e_tiles = emit_scores_ctx(lg, h, mqs_by_lg[lg][h])
                        if pending is not None:
                            emit_epilogue(*pending)
                            if pending[1] == 1:
                                emit_dense(pending[0])
                        pending = (lg, h, pc, e_tiles)
                emit_epilogue(*pending, tail=True)
                emit_dense(LG - 1)

    _split_multi_waits(nc)
    return nc


_NC_CACHE = None


def _get_program():
    global _NC_CACHE
    if _NC_CACHE is None:
        _NC_CACHE = _build_program()
    return _NC_CACHE


def kernel(queries, keys, values, attn_mask, Wq, Wk, Wv, bv, Wcb, mixing, Wd, bd):
    queries = np.asarray(queries, np.float32)
    keys = np.asarray(keys, np.float32)
    values = np.asarray(values, np.float32)
    Wq = np.asarray(Wq, np.float32)
    Wk = np.asarray(Wk, np.float32)
    Wv = np.asarray(Wv, np.float32)
    bv = np.asarray(bv, np.float32)
    Wcb = np.asarray(Wcb, np.float32)
    mixing = np.asarray(mixing, np.float32)
    Wd = np.asarray(Wd, np.float32)
    bd = np.asarray(bd, np.float32)

    WqT = np.ascontiguousarray(Wq.T)
    WkT = np.ascontiguousarray(Wk.T)
    WvT_full = np.ascontiguousarray(Wv.T)          # [d, v]
    WcbT = np.ascontiguousarray(Wcb.T)             # [d, h]
    WdT_full = np.ascontiguousarray(Wd.T)          # [v, o]
    mixT = np.ascontiguousarray(mixing.T)          # [e, h]

    in_maps = []
    for core in range(8):
        b, p = divmod(core, PAIRS)
        vcols = np.zeros((D, 256), np.float32)
        vcols[:, 0:128] = WvT_full[:, 128 * p : 128 * p + 128]
        in_maps.append(
            {
                "qT": np.ascontiguousarray(queries[b].T),
                "kTin": np.ascontiguousarray(keys[b].T),
                "vTin": np.ascontiguousarray(values[b].T),
                "WqT": WqT,
                "WkT": WkT,
                "WvT": vcols,
                "WcbT": np.ascontiguousarray(WcbT[:, 2 * p : 2 * p + 2]),
                "mixT": np.ascontiguousarray(mixT[:, 2 * p : 2 * p + 2]),
                "WdT": np.ascontiguousarray(
                    WdT_full[128 * p : 128 * p + 128].reshape(2, HD, D)
                ),
                "ones": np.ones((128, 128), np.float32),
            }
        )

    nc = _get_program()
    trace = bool(int(os.environ.get("KERNEL_TRACE", "0")))
    trace_dir = os.environ.get("KERNEL_TRACE_DIR") or None
    res = run_bass_kernel_spmd(
        nc, in_maps, list(range(8)), trace=trace, tmpdir=trace_dir
    )
    if trace and res.exec_time_ns is not None:
        print(f"HW exec time: {res.exec_time_ns} ns")
        kernel.last_exec_time_ns = res.exec_time_ns

    out = np.empty((B, L, D), np.float32)
    probs = np.empty((B, H, L, S), np.float32)
    bias = bd + Wd @ bv  # bv folds into out because softmax rows sum to 1
    for b in range(B):
        acc = np.zeros((D, L), np.float32)
        for p in range(PAIRS):
            r = res.results[4 * b + p]
            acc += r["outT"]
            probs[b, 2 * p] = r["probsT"][0].T
            probs[b, 2 * p + 1] = r["probsT"][1].T
        out[b] = acc.T + bias
    return out, probs


kernel.last_exec_time_ns = None
